# revision 2
# baseline (speedup 1.0000x reference)
"""Debayer3x3 Trainium2 Bass kernel.

Full inputs -> full output. Internally: data-parallel over 8 NeuronCores,
each core processes half an image (1080 rows) with a 1-pixel halo.

Math (BG-layout bilinear debayer), verified against the reference:
  c0 = x (identity), c1 = 0.25*(U+D+L+R), c2 = 0.25*(diagonals),
  c3 = 0.5*(L+R), c4 = 0.5*(U+D)
  R = [[c0, c3], [c4, c2]]  (2x2 parity pattern, (row%2, col%2))
  G = [[c1, c0], [c0, c1]]
  B = [[c2, c4], [c3, c0]]

On-core layout: each SBUF partition owns a block of R=10 consecutive output
rows plus 2 halo rows (compute engines cannot read partition-shifted
operands, so all vertical neighbors must live in the same partition's free
dim). 1080 rows = 108 partitions x 10 rows. DVE computes shared sums
(Hs = L+R, Vs = U+D, diag = Vs-of-Hs, cross = Hs+Vs), ACT (scalar engine)
assembles the 12 (channel x parity) quadrants with the 0.5/0.25 scales
fused into the copies.
"""

import sys
from contextlib import ExitStack

import numpy as np

if "/opt/trn_rl_repo" not in sys.path:
    sys.path.insert(0, "/opt/trn_rl_repo")

import concourse.bacc as bacc
import concourse.bass as bass
import concourse.mybir as mybir
import concourse.tile as tile
from concourse.bass_utils import run_bass_kernel_spmd

B, H, W = 4, 2160, 3840
HALF = H // 2  # 1080 rows per core
N_CORES = 8
RB = 10  # output rows per partition (must be even; RB * n_part == rows)

F32 = mybir.dt.float32


def build_program(n_part, width, chunk, num_devices=N_CORES):
    """Build the per-core SPMD program.

    Input  "x": (RB*n_part + 2, width + 2)  shard with 1-px halo on all sides
    Output "y": (3, RB*n_part, width)
    """
    rows = RB * n_part
    SW = width + 2  # shard row stride
    nc = bacc.Bacc(
        "TRN2",
        target_bir_lowering=False,
        debug=False,
        enable_asserts=True,
        num_devices=num_devices,
    )
    x = nc.dram_tensor("x", (rows + 2, SW), F32, kind="ExternalInput")
    y = nc.dram_tensor("y", (3, rows, width), F32, kind="ExternalOutput")

    assert width % chunk == 0 and chunk % 2 == 0
    n_chunks = width // chunk

    with tile.TileContext(nc) as tc:
        with ExitStack() as ctx:
            inp = ctx.enter_context(tc.tile_pool(name="inp", bufs=2))
            mid = ctx.enter_context(tc.tile_pool(name="mid", bufs=1))
            outp = ctx.enter_context(tc.tile_pool(name="outp", bufs=2))
            for c in range(n_chunks):
                _emit_tile(nc, inp, mid, outp, x, y, n_part, width, c * chunk, chunk)

    nc.compile()
    return nc


def _emit_tile(nc, inp, mid, outp, x, y, NP, width, c0, CW):
    """One tile: all NP partition row-blocks x CW output columns at col c0."""
    CH = CW // 2
    SW = width + 2
    rows = RB * NP

    # Input tile: partition p holds shard rows RB*p .. RB*p+11 (= image rows
    # RB*p-1 .. RB*p+10), shard cols c0 .. c0+CW+1 (= image cols c0-1..c0+CW).
    tin = inp.tile([NP, RB + 2, CW + 2], F32, tag="tin")
    src = bass.AP(x, c0, [[RB * SW, NP], [SW, RB + 2], [1, CW + 2]])
    nc.sync.dma_start(tin[:], src)

    # Row index k in tin/Hs is image row RB*p + k - 1; image col j is k-col j+1.
    # Hs[p,k,j] = x(row, j-1) + x(row, j+1) at image col j, all RB+2 rows.
    Hs = mid.tile([NP, RB + 2, CW], F32, tag="Hs")
    nc.vector.tensor_add(Hs[:], tin[:, :, 0:CW], tin[:, :, 2 : CW + 2])

    # Vs[p,t,j] = up + down of output row t (t = 0..RB-1): tin rows t, t+2.
    Vs = mid.tile([NP, RB, CW], F32, tag="Vs")
    nc.vector.tensor_add(
        Vs[:], tin[:, 0:RB, 1 : CW + 1], tin[:, 2 : RB + 2, 1 : CW + 1]
    )

    # Diagonal sums: rows t-1,t+1 of Hs = Hs rows t, t+2.
    # Needed at even rows/even cols (B) and odd rows/odd cols (R).
    Dse = mid.tile([NP, RB // 2, CH], F32, tag="Dse")
    nc.vector.tensor_add(
        Dse[:], Hs[:, 0:RB:2, 0:CW:2], Hs[:, 2 : RB + 2 : 2, 0:CW:2]
    )
    Dso = mid.tile([NP, RB // 2, CH], F32, tag="Dso")
    nc.vector.tensor_add(
        Dso[:], Hs[:, 1 : RB + 1 : 2, 1:CW:2], Hs[:, 3 : RB + 2 : 2, 1:CW:2]
    )

    # Cross sums = Hs + Vs at the output row (Hs row t+1): even/even and
    # odd/odd (both for G).
    S4e = mid.tile([NP, RB // 2, CH], F32, tag="S4e")
    nc.vector.tensor_add(
        S4e[:], Hs[:, 1 : RB + 1 : 2, 0:CW:2], Vs[:, 0:RB:2, 0:CW:2]
    )
    S4o = mid.tile([NP, RB // 2, CH], F32, tag="S4o")
    nc.vector.tensor_add(
        S4o[:], Hs[:, 2 : RB + 2 : 2, 1:CW:2], Vs[:, 1:RB:2, 1:CW:2]
    )

    # Assemble interleaved output tiles; scales fused into ACT copies.
    tR = outp.tile([NP, RB, CW], F32, tag="tR")
    tG = outp.tile([NP, RB, CW], F32, tag="tG")
    tB = outp.tile([NP, RB, CW], F32, tag="tB")

    ev, od = slice(0, RB, 2), slice(1, RB, 2)  # output row parities
    ec, oc = slice(0, CW, 2), slice(1, CW, 2)  # output col parities
    # x at output row t, col j  ->  tin[:, t+1, j+1]
    x_ev = slice(1, RB + 1, 2)  # tin rows for even output rows
    x_od = slice(2, RB + 2, 2)  # tin rows for odd output rows

    # R: [[x, 0.5*Hs], [0.5*Vs, 0.25*diag]]
    nc.scalar.copy(tR[:, ev, ec], tin[:, x_ev, 1 : CW + 1 : 2])
    nc.scalar.mul(tR[:, ev, oc], Hs[:, x_ev, oc], 0.5)
    nc.scalar.mul(tR[:, od, ec], Vs[:, od, ec], 0.5)
    nc.scalar.mul(tR[:, od, oc], Dso[:], 0.25)
    # G: [[0.25*cross, x], [x, 0.25*cross]]
    nc.scalar.mul(tG[:, ev, ec], S4e[:], 0.25)
    nc.scalar.copy(tG[:, ev, oc], tin[:, x_ev, 2 : CW + 2 : 2])
    nc.scalar.copy(tG[:, od, ec], tin[:, x_od, 1 : CW + 1 : 2])
    nc.scalar.mul(tG[:, od, oc], S4o[:], 0.25)
    # B: [[0.25*diag, 0.5*Vs], [0.5*Hs, x]]
    nc.scalar.mul(tB[:, ev, ec], Dse[:], 0.25)
    nc.scalar.mul(tB[:, ev, oc], Vs[:, ev, oc], 0.5)
    nc.scalar.mul(tB[:, od, ec], Hs[:, x_od, ec], 0.5)
    nc.scalar.copy(tB[:, od, oc], tin[:, x_od, 2 : CW + 2 : 2])

    for ci, tch in enumerate((tR, tG, tB)):
        dst = bass.AP(
            y, ci * rows * width + c0, [[RB * width, NP], [width, RB], [1, CW]]
        )
        nc.sync.dma_start(dst, tch[:])


_PROGRAM = None


def _get_program():
    global _PROGRAM
    if _PROGRAM is None:
        _PROGRAM = build_program(n_part=HALF // RB, width=W, chunk=384)
    return _PROGRAM


def _shards(x):
    """x: (4, 1, 2160, 3840) -> 8 halo'd shards of (1082, 3842)."""
    xp = np.pad(np.asarray(x)[:, 0], ((0, 0), (1, 1), (1, 1)), mode="edge")
    maps = []
    for c in range(N_CORES):
        b, h = divmod(c, 2)
        maps.append(
            {"x": np.ascontiguousarray(xp[b, h * HALF : h * HALF + HALF + 2, :])}
        )
    return maps


def kernel(x, kernels=None, index=None, _trace=False):
    nc = _get_program()
    in_maps = _shards(x)
    res = run_bass_kernel_spmd(
        nc, in_maps, core_ids=list(range(N_CORES)), trace=_trace
    )
    out = np.empty((B, 3, H, W), np.float32)
    for c in range(N_CORES):
        b, h = divmod(c, 2)
        out[b, :, h * HALF : (h + 1) * HALF, :] = res.results[c]["y"]
    if _trace:
        kernel.last_exec_time_ns = res.exec_time_ns
        kernel.last_results = res
    return out


# revision 3
# speedup vs baseline: 1.1210x; 1.1210x over previous
"""Debayer3x3 Trainium2 Bass kernel.

Full inputs -> full output. Internally: data-parallel over 8 NeuronCores,
each core processes half an image (1080 rows) with a 1-pixel halo.

Math (BG-layout bilinear debayer), verified against the reference:
  c0 = x (identity), c1 = 0.25*(U+D+L+R), c2 = 0.25*(diagonals),
  c3 = 0.5*(L+R), c4 = 0.5*(U+D)
  R = [[c0, c3], [c4, c2]]  (2x2 parity pattern, (row%2, col%2))
  G = [[c1, c0], [c0, c1]]
  B = [[c2, c4], [c3, c0]]

On-core layout: each SBUF partition owns a block of R=10 consecutive output
rows plus 2 halo rows (compute engines cannot read partition-shifted
operands, so all vertical neighbors must live in the same partition's free
dim). 1080 rows = 108 partitions x 10 rows. DVE computes shared sums
(Hs = L+R, Vs = U+D, diag = Vs-of-Hs, cross = Hs+Vs), ACT (scalar engine)
assembles the 12 (channel x parity) quadrants with the 0.5/0.25 scales
fused into the copies.
"""

import sys
from contextlib import ExitStack

import numpy as np

if "/opt/trn_rl_repo" not in sys.path:
    sys.path.insert(0, "/opt/trn_rl_repo")

import concourse.bacc as bacc
import concourse.bass as bass
import concourse.mybir as mybir
import concourse.tile as tile
from concourse.bass_utils import run_bass_kernel_spmd

B, H, W = 4, 2160, 3840
HALF = H // 2  # 1080 rows per core
N_CORES = 8
RB = 10  # output rows per partition (must be even; RB * n_part == rows)

F32 = mybir.dt.float32


def build_program(n_part, width, chunk, num_devices=N_CORES):
    """Build the per-core SPMD program.

    Input  "x": (RB*n_part + 2, width + 2)  shard with 1-px halo on all sides
    Output "y": (3, RB*n_part, width)
    """
    rows = RB * n_part
    SW = width + 2  # shard row stride
    nc = bacc.Bacc(
        "TRN2",
        target_bir_lowering=False,
        debug=False,
        enable_asserts=True,
        num_devices=num_devices,
    )
    x = nc.dram_tensor("x", (rows + 2, SW), F32, kind="ExternalInput")
    y = nc.dram_tensor("y", (3, rows, width), F32, kind="ExternalOutput")

    assert width % chunk == 0 and chunk % 2 == 0
    n_chunks = width // chunk

    with tile.TileContext(nc) as tc:
        with ExitStack() as ctx:
            inp = ctx.enter_context(tc.tile_pool(name="inp", bufs=2))
            mid = ctx.enter_context(tc.tile_pool(name="mid", bufs=1))
            outp = ctx.enter_context(tc.tile_pool(name="outp", bufs=2))
            for c in range(n_chunks):
                _emit_tile(nc, inp, mid, outp, x, y, n_part, width, c * chunk, chunk)

    nc.compile()
    return nc


def _emit_tile(nc, inp, mid, outp, x, y, NP, width, c0, CW):
    """One tile: all NP partition row-blocks x CW output columns at col c0."""
    CH = CW // 2
    SW = width + 2
    rows = RB * NP

    # Input tile: partition p holds shard rows RB*p .. RB*p+11 (= image rows
    # RB*p-1 .. RB*p+10), shard cols c0 .. c0+CW+1 (= image cols c0-1..c0+CW).
    tin = inp.tile([NP, RB + 2, CW + 2], F32, tag="tin")
    src = bass.AP(x, c0, [[RB * SW, NP], [SW, RB + 2], [1, CW + 2]])
    nc.sync.dma_start(tin[:], src)

    # Row index k in tin/Hs is image row RB*p + k - 1; image col j is k-col j+1.
    # Hs[p,k,j] = x(row, j-1) + x(row, j+1) at image col j, all RB+2 rows.
    Hs = mid.tile([NP, RB + 2, CW], F32, tag="Hs")
    nc.vector.tensor_add(Hs[:], tin[:, :, 0:CW], tin[:, :, 2 : CW + 2])

    # Vs[p,t,j] = up + down of output row t (t = 0..RB-1): tin rows t, t+2.
    Vs = mid.tile([NP, RB, CW], F32, tag="Vs")
    nc.vector.tensor_add(
        Vs[:], tin[:, 0:RB, 1 : CW + 1], tin[:, 2 : RB + 2, 1 : CW + 1]
    )

    # Diagonal sums: rows t-1,t+1 of Hs = Hs rows t, t+2.
    # Needed at even rows/even cols (B) and odd rows/odd cols (R).
    Dse = mid.tile([NP, RB // 2, CH], F32, tag="Dse")
    nc.vector.tensor_add(
        Dse[:], Hs[:, 0:RB:2, 0:CW:2], Hs[:, 2 : RB + 2 : 2, 0:CW:2]
    )
    Dso = mid.tile([NP, RB // 2, CH], F32, tag="Dso")
    nc.vector.tensor_add(
        Dso[:], Hs[:, 1 : RB + 1 : 2, 1:CW:2], Hs[:, 3 : RB + 2 : 2, 1:CW:2]
    )

    # Cross sums = Hs + Vs at the output row (Hs row t+1): even/even and
    # odd/odd (both for G).
    S4e = mid.tile([NP, RB // 2, CH], F32, tag="S4e")
    nc.vector.tensor_add(
        S4e[:], Hs[:, 1 : RB + 1 : 2, 0:CW:2], Vs[:, 0:RB:2, 0:CW:2]
    )
    S4o = mid.tile([NP, RB // 2, CH], F32, tag="S4o")
    nc.vector.tensor_add(
        S4o[:], Hs[:, 2 : RB + 2 : 2, 1:CW:2], Vs[:, 1:RB:2, 1:CW:2]
    )

    # Assemble interleaved output tiles; scales fused into ACT copies.
    tR = outp.tile([NP, RB, CW], F32, tag="tR")
    tG = outp.tile([NP, RB, CW], F32, tag="tG")
    tB = outp.tile([NP, RB, CW], F32, tag="tB")

    ev, od = slice(0, RB, 2), slice(1, RB, 2)  # output row parities
    ec, oc = slice(0, CW, 2), slice(1, CW, 2)  # output col parities
    # x at output row t, col j  ->  tin[:, t+1, j+1]
    x_ev = slice(1, RB + 1, 2)  # tin rows for even output rows
    x_od = slice(2, RB + 2, 2)  # tin rows for odd output rows

    # R: [[x, 0.5*Hs], [0.5*Vs, 0.25*diag]]
    nc.scalar.copy(tR[:, ev, ec], tin[:, x_ev, 1 : CW + 1 : 2])
    nc.scalar.mul(tR[:, ev, oc], Hs[:, x_ev, oc], 0.5)
    nc.scalar.mul(tR[:, od, ec], Vs[:, od, ec], 0.5)
    nc.scalar.mul(tR[:, od, oc], Dso[:], 0.25)
    # G: [[0.25*cross, x], [x, 0.25*cross]]
    nc.scalar.mul(tG[:, ev, ec], S4e[:], 0.25)
    nc.scalar.copy(tG[:, ev, oc], tin[:, x_ev, 2 : CW + 2 : 2])
    nc.scalar.copy(tG[:, od, ec], tin[:, x_od, 1 : CW + 1 : 2])
    nc.scalar.mul(tG[:, od, oc], S4o[:], 0.25)
    # B: [[0.25*diag, 0.5*Vs], [0.5*Hs, x]]
    nc.scalar.mul(tB[:, ev, ec], Dse[:], 0.25)
    nc.scalar.mul(tB[:, ev, oc], Vs[:, ev, oc], 0.5)
    nc.scalar.mul(tB[:, od, ec], Hs[:, x_od, ec], 0.5)
    nc.scalar.copy(tB[:, od, oc], tin[:, x_od, 2 : CW + 2 : 2])

    # Spread output stores across the three DMA-issue paths (SP + ACT HWDGE
    # rings, GpSimd SWDGE) so the SDMA engines keep multiple queues in
    # flight; the input load stays on nc.sync.
    parity = (c0 // CW) % 2
    b_eng = nc.scalar if parity else nc.sync
    for eng, ci, tch in ((nc.scalar, 0, tR), (nc.gpsimd, 1, tG), (b_eng, 2, tB)):
        dst = bass.AP(
            y, ci * rows * width + c0, [[RB * width, NP], [width, RB], [1, CW]]
        )
        eng.dma_start(dst, tch[:])


_PROGRAM = None


def _get_program():
    global _PROGRAM
    if _PROGRAM is None:
        _PROGRAM = build_program(n_part=HALF // RB, width=W, chunk=384)
    return _PROGRAM


def _shards(x):
    """x: (4, 1, 2160, 3840) -> 8 halo'd shards of (1082, 3842)."""
    xp = np.pad(np.asarray(x)[:, 0], ((0, 0), (1, 1), (1, 1)), mode="edge")
    maps = []
    for c in range(N_CORES):
        b, h = divmod(c, 2)
        maps.append(
            {"x": np.ascontiguousarray(xp[b, h * HALF : h * HALF + HALF + 2, :])}
        )
    return maps


def kernel(x, kernels=None, index=None, _trace=False):
    nc = _get_program()
    in_maps = _shards(x)
    res = run_bass_kernel_spmd(
        nc, in_maps, core_ids=list(range(N_CORES)), trace=_trace
    )
    out = np.empty((B, 3, H, W), np.float32)
    for c in range(N_CORES):
        b, h = divmod(c, 2)
        out[b, :, h * HALF : (h + 1) * HALF, :] = res.results[c]["y"]
    if _trace:
        kernel.last_exec_time_ns = res.exec_time_ns
        kernel.last_results = res
    return out
